# revision 14
# baseline (speedup 1.0000x reference)
"""Trainium2 Bass kernel for nn_CPAMDec_Mix (dual cross-attention mix block).

Math (per batch b):
    q1 = wq1 @ x1      q2 = wq2 @ x2          (1x1 convs, [128, N] each)
    qT = concat(q1, q2) on channel -> [256, N]
    k_sT = w k_s @ y_s^T                      ([256, K])
    v_s  = y_s @ wv_s^T                       ([K, C])
    e_sT[k, n] = sum_d k_sT[d, k] qT[d, n]    ([K, N])
    attnT = softmax_k(|e1T - e2T|)            (softmax over k, no max-sub:
                                               |e| <= ~40 << 88 overflow)
    out_s = scale * (v_s^T @ attnT + bv_s) + x_s

Sharding: data-parallel over batch B=16 across 8 cores (2 batches/core),
weights replicated. Everything stays in [c, n] layout so DRAM I/O is
contiguous; softmax lives in [k, n] layout so no transposes are needed
(k-sum via ones-matmul, 1/sum broadcast comes out of the same matmul).
"""

import os
import numpy as np

import concourse.mybir as mybir
import concourse.tile as tile
from concourse import bacc
from concourse.bass import ts
from concourse.bass_utils import run_bass_kernel_spmd

F32 = mybir.dt.float32
F32R = mybir.dt.float32r
AF = mybir.ActivationFunctionType
ALU = mybir.AluOpType

B, C, WH, K = 16, 512, 4096, 128
NCORES = 8
BPC = B // NCORES          # batches per core
D = 128                    # per-stream q channels (C // 4)
NT = 512                   # n-tile size
NTILES = WH // NT          # 8
CCH = C // 128             # 4 c-chunks

# Matmul input dtype knobs (float32r = single-pass PE fp32, 4x faster at
# moving-dim >= 256; float32 = 2-pass full precision).
DT_Q = F32R   # q projections (also x-tile dtype)
DT_E = F32    # attention logits (kT / q tile dtype)
DT_V = F32R   # v + k projections (y / wk / wv tile dtype)
DT_O = F32R   # output projection (vv / attnt tile dtype)
DT_S = F32R   # softmax denominator ones-matmul (ones / expt tile dtype)

_PROGRAM = None
LAST_RESULTS = None


def _body(tc, io):
    nc = tc.nc
    from contextlib import ExitStack

    with ExitStack() as ctx:
        def _bufs(name, default):
            return int(os.environ.get(f"KM_BUFS_{name}", default))

        consts = ctx.enter_context(tc.tile_pool(name="consts", bufs=1))
        bpool = ctx.enter_context(tc.tile_pool(name="batch", bufs=2))
        xpool = ctx.enter_context(tc.tile_pool(name="xs", bufs=_bufs("X", 4)))
        qpool = ctx.enter_context(tc.tile_pool(name="qs", bufs=_bufs("Q", 2)))
        spool = ctx.enter_context(tc.tile_pool(name="soft", bufs=_bufs("S", 2)))
        opool = ctx.enter_context(tc.tile_pool(name="outs", bufs=_bufs("O", 3)))
        pq = ctx.enter_context(tc.tile_pool(name="pq", bufs=_bufs("PQ", 1), space="PSUM"))
        pe = ctx.enter_context(tc.tile_pool(name="pe", bufs=_bufs("PE", 2), space="PSUM"))
        psb = ctx.enter_context(tc.tile_pool(name="psb", bufs=1, space="PSUM"))
        po = ctx.enter_context(tc.tile_pool(name="po", bufs=_bufs("PO", 4), space="PSUM"))

        # ---- constants (weights replicated per core) ----
        wq_sb, wk_sb, wv_sb, bq_sb, bk_sb, sbv_sb = {}, {}, {}, {}, {}, {}
        for s in (1, 2):
            wq_sb[s] = consts.tile([128, CCH, D], DT_Q, tag=f"wq{s}", name=f"wq{s}")
            nc.sync.dma_start(wq_sb[s][:], io[f"wq{s}t"][:].bitcast(DT_Q))
            wk_sb[s] = consts.tile([128, CCH, 2 * D], DT_V, tag=f"wk{s}", name=f"wk{s}")
            nc.sync.dma_start(wk_sb[s][:], io[f"wk{s}t"][:].bitcast(DT_V))
            wv_sb[s] = consts.tile([128, CCH, C], DT_V, tag=f"wv{s}", name=f"wv{s}")
            nc.sync.dma_start(wv_sb[s][:], io[f"wv{s}t"][:].bitcast(DT_V))
            bq_sb[s] = consts.tile([128, 1], F32, tag=f"bq{s}", name=f"bq{s}")
            nc.sync.dma_start(bq_sb[s][:], io[f"bq{s}"][:])
            bk_sb[s] = consts.tile([128, 2], F32, tag=f"bk{s}", name=f"bk{s}")
            nc.sync.dma_start(bk_sb[s][:], io[f"bk{s}"][:])
            sbv_sb[s] = consts.tile([128, CCH], F32, tag=f"sbv{s}", name=f"sbv{s}")
            nc.sync.dma_start(sbv_sb[s][:], io[f"sbv{s}"][:])
        scale_sb = consts.tile([128, 1], F32, tag="scale")
        nc.sync.dma_start(scale_sb[:], io["scale_rep"][:])
        ones_sb = consts.tile([128, 128], DT_S, tag="ones")
        nc.sync.dma_start(ones_sb[:], io["ones"][:].bitcast(DT_S))

        for _rep in range(int(os.environ.get("KM_REPEAT", 1))):
         for b in range(BPC):
            # ---- per-batch: k_sT [128, 2, 128] and v_s [128, C] ----
            kT, vv = {}, {}
            for s in (1, 2):
                yt = bpool.tile([128, CCH, K], DT_V, tag=f"y{s}", name=f"y{s}")
                nc.sync.dma_start(yt[:], io[f"y{s}t"][b].bitcast(DT_V))
                kT[s] = bpool.tile([128, 2, K], DT_E, tag=f"k{s}", name=f"k{s}")
                for dc in range(2):
                    pk = pe.tile([128, NT], F32, tag="pe1", name="pk")[:, :K]
                    for cc in range(CCH):
                        nc.tensor.matmul(
                            pk[:],
                            wk_sb[s][:, cc, ts(dc, D)],
                            yt[:, cc, :],
                            start=(cc == 0),
                            stop=(cc == CCH - 1),
                        )
                    nc.scalar.activation(
                        kT[s][:, dc, :], pk[:], AF.Identity,
                        bias=bk_sb[s][:, dc : dc + 1],
                    )
                vv[s] = bpool.tile([128, C], DT_O, tag=f"v{s}", name=f"v{s}")
                pv = po.tile([128, NT], F32, tag="po", name="pv")
                for cc in range(CCH):
                    nc.tensor.matmul(
                        pv[:],
                        yt[:, cc, :],
                        wv_sb[s][:, cc, :],
                        start=(cc == 0),
                        stop=(cc == CCH - 1),
                    )
                # v_s scaled by `scale` here; bias bv folded into the output
                # residual (attn rows sum to 1).
                nc.scalar.activation(
                    vv[s][:], pv[:], AF.Copy, bias=0.0, scale=scale_sb[:],
                )

            x_ap = {s: io[f"x{s}"][b].rearrange("(co p) n -> p co n", p=128)
                    for s in (1, 2)}
            o_ap = {s: io[f"out{s}"][b].rearrange("(co p) n -> p co n", p=128)
                    for s in (1, 2)}

            for nt in range(int(os.environ.get("KM_NTILES", NTILES))):
                nsl = ts(nt, NT)
                # ---- load x tiles ----
                xt = {}
                for s in (1, 2):
                    xt[s] = xpool.tile([128, CCH, NT], DT_Q, tag=f"x{s}", name=f"x{s}")
                    nc.sync.dma_start(xt[s][:], x_ap[s][:, :, nsl].bitcast(DT_Q))

                # ---- q projections: qT chunk s -> [128, NT] ----
                q = {}
                for s in (1, 2):
                    pqt = pq.tile([128, NT], F32, tag="pq", name="pqt")
                    for cc in range(CCH):
                        nc.tensor.matmul(
                            pqt[:],
                            wq_sb[s][:, cc, :],
                            xt[s][:, cc, :],
                            start=(cc == 0),
                            stop=(cc == CCH - 1),
                        )
                    q[s] = qpool.tile([128, NT], DT_E, tag=f"q{s}", name=f"q{s}")
                    nc.scalar.activation(
                        q[s][:], pqt[:], AF.Identity, bias=bq_sb[s][:],
                    )

                # ---- attention logits eT_s [k, n] ----
                pes = {}
                for s in (1, 2):
                    pes[s] = pe.tile([128, NT], F32, tag="pe1", name=f"pe{s}")
                    for dc in range(2):
                        nc.tensor.matmul(
                            pes[s][:],
                            kT[s][:, dc, :],
                            q[dc + 1][:],
                            start=(dc == 0),
                            stop=(dc == 1),
                        )

                # ---- softmax over k (partition dim), no max subtraction ----
                # (DVE reads at most one PSUM operand: stage e1 via ACT)
                e1sb = spool.tile([128, NT], F32, tag="e1sb")
                nc.scalar.activation(e1sb[:], pes[1][:], AF.Copy)
                diff = spool.tile([128, NT], F32, tag="diff")
                nc.vector.tensor_sub(diff[:], e1sb[:], pes[2][:])
                adiff = spool.tile([128, NT], F32, tag="adiff")
                nc.scalar.activation(adiff[:], diff[:], AF.Abs)
                expt = spool.tile([128, NT], DT_S, tag="expt")
                nc.scalar.activation(expt[:], adiff[:], AF.Exp)
                # all-partition sum broadcast: ones[128,128]^T @ expt
                psum_s = psb.tile([128, NT], F32, tag="psb", name="psum_s")
                nc.tensor.matmul(
                    psum_s[:], ones_sb[:], expt[:],
                )
                rb = spool.tile([128, NT], F32, tag="rb")
                nc.vector.reciprocal(rb[:], psum_s[:])
                attnt = spool.tile([128, NT], DT_O, tag="attnt")
                nc.vector.tensor_mul(attnt[:], expt[:], rb[:])

                # ---- outputs: out_s[c, n] = v_s^T @ attnT + s*bv_s + x_s ----
                for s in (1, 2):
                    ot = opool.tile([128, CCH, NT], F32, tag=f"o{s}", name=f"o{s}")
                    for cc in range(CCH):
                        pot = po.tile([128, NT], F32, tag="po", name="pot")
                        nc.tensor.matmul(
                            pot[:],
                            vv[s][:, ts(cc, 128)],
                            attnt[:],
                        )
                        nc.vector.scalar_tensor_tensor(
                            ot[:, cc, :],
                            pot[:],
                            sbv_sb[s][:, cc : cc + 1],
                            xt[s][:, cc, :],
                            op0=ALU.add,
                            op1=ALU.add,
                        )
                    nc.sync.dma_start(o_ap[s][:, :, nsl], ot[:])


def build_program():
    nc = bacc.Bacc(
        "TRN2", target_bir_lowering=False, debug=False, enable_asserts=False,
    )
    io = {}

    def din(name, shape):
        io[name] = nc.dram_tensor(name, shape, F32, kind="ExternalInput").ap()

    def dout(name, shape):
        io[name] = nc.dram_tensor(name, shape, F32, kind="ExternalOutput").ap()

    din("x1", [BPC, C, WH])
    din("x2", [BPC, C, WH])
    din("y1t", [BPC, 128, CCH, K])
    din("y2t", [BPC, 128, CCH, K])
    for s in (1, 2):
        din(f"wq{s}t", [128, CCH, D])
        din(f"wk{s}t", [128, CCH, 2 * D])
        din(f"wv{s}t", [128, CCH, C])
        din(f"bq{s}", [128, 1])
        din(f"bk{s}", [128, 2])
        din(f"sbv{s}", [128, CCH])
    din("scale_rep", [128, 1])
    din("ones", [128, 128])
    dout("out1", [BPC, C, WH])
    dout("out2", [BPC, C, WH])

    with tile.TileContext(nc) as tc:
        _body(tc, io)
    nc.compile()
    return nc


def _get_program():
    global _PROGRAM
    if _PROGRAM is None:
        _PROGRAM = build_program()
    return _PROGRAM


def _to_chunked(w):
    # host weight [out, in] -> transposed chunked SBUF layout [p, co, out]
    # (wT[c, out] with input-channel c = co*128 + p), contiguous for DMA
    out_dim, in_dim = w.shape
    return np.ascontiguousarray(
        w.T.reshape(in_dim // 128, 128, out_dim).transpose(1, 0, 2)
    )


def _bias_chunks(bv):
    # [d] -> [128, d//128] with d = dc*128 + p
    return np.ascontiguousarray(bv.reshape(-1, 128).T)


def prepare_in_maps(inputs):
    f = lambda a: np.ascontiguousarray(np.asarray(a, dtype=np.float32))
    x1 = f(inputs["x1"]).reshape(B, C, WH)
    x2 = f(inputs["x2"]).reshape(B, C, WH)
    # y^T per batch in chunked layout [b, p, co, k]
    def yt_chunk(y):
        ytr = f(y).transpose(0, 2, 1)  # [B, C, K]
        return np.ascontiguousarray(
            ytr.reshape(B, CCH, 128, K).transpose(0, 2, 1, 3)
        )
    y1t = yt_chunk(inputs["y1"])
    y2t = yt_chunk(inputs["y2"])
    scale = float(np.asarray(inputs["scale"]).reshape(-1)[0])

    shared = {"scale_rep": np.full((128, 1), scale, np.float32),
              "ones": np.ones((128, 128), np.float32)}
    for s in (1, 2):
        shared[f"wq{s}t"] = _to_chunked(f(inputs[f"wq{s}"]))
        shared[f"wk{s}t"] = _to_chunked(f(inputs[f"wk{s}"]))
        shared[f"wv{s}t"] = _to_chunked(f(inputs[f"wv{s}"]))
        shared[f"bq{s}"] = f(inputs[f"bq{s}"]).reshape(128, 1)
        shared[f"bk{s}"] = _bias_chunks(f(inputs[f"bk{s}"]))
        shared[f"sbv{s}"] = _bias_chunks(scale * f(inputs[f"bv{s}"]))

    in_maps = []
    for c in range(NCORES):
        sl = slice(BPC * c, BPC * (c + 1))
        in_maps.append({
            "x1": np.ascontiguousarray(x1[sl]),
            "x2": np.ascontiguousarray(x2[sl]),
            "y1t": np.ascontiguousarray(y1t[sl]),
            "y2t": np.ascontiguousarray(y2t[sl]),
            **shared,
        })
    return in_maps


def kernel(**inputs):
    global LAST_RESULTS
    nc = _get_program()
    in_maps = prepare_in_maps(inputs)
    try:
        res = run_bass_kernel_spmd(nc, in_maps, list(range(NCORES)))
    except Exception:
        # transient NRT device hiccups have been observed; retry once
        res = run_bass_kernel_spmd(nc, in_maps, list(range(NCORES)))
    LAST_RESULTS = res
    out1 = np.concatenate(
        [res.results[c]["out1"] for c in range(NCORES)], axis=0
    ).reshape(B, C, 64, 64)
    out2 = np.concatenate(
        [res.results[c]["out2"] for c in range(NCORES)], axis=0
    ).reshape(B, C, 64, 64)
    return out1, out2


def bench(inputs, iters=30, repeat=1, nc=None):
    """Time warm back-to-back executions of the compiled NEFF on 8 cores.

    Replicates run_bass_via_pjrt's shard_map jit, but without output-buffer
    donation so device-resident inputs can be reused across calls (this
    kernel writes every output element, so uninitialized result buffers are
    fine). Returns (per_call_seconds, results_list).
    """
    import time as _time
    import jax
    import concourse.mybir as _mybir
    from jax.experimental.shard_map import shard_map
    from jax.sharding import Mesh, PartitionSpec
    from concourse.bass2jax import _bass_exec_p, install_neuronx_cc_hook

    from concourse.bass2jax import partition_id_tensor
    install_neuronx_cc_hook()
    if nc is None:
        nc = _get_program()
    in_maps = prepare_in_maps(inputs)

    partition_name = nc.partition_id_tensor.name if nc.partition_id_tensor else None
    in_names, out_names, out_avals = [], [], []
    for alloc in nc.m.functions[0].allocations:
        if not isinstance(alloc, _mybir.MemoryLocationSet):
            continue
        name = alloc.memorylocations[0].name
        if alloc.kind == "ExternalInput":
            if name != partition_name:
                in_names.append(name)
        elif alloc.kind == "ExternalOutput":
            out_names.append(name)
            out_avals.append(jax.core.ShapedArray(
                tuple(alloc.tensor_shape), _mybir.dt.np(alloc.dtype)))
    n_params = len(in_names)
    all_names = in_names + out_names
    if partition_name is not None:
        all_names = all_names + [partition_name]

    def _call(ins, bufs):
        operands = list(ins) + list(bufs)
        if partition_name is not None:
            operands.append(partition_id_tensor())
        return tuple(_bass_exec_p.bind(
            *operands,
            out_avals=tuple(out_avals),
            in_names=tuple(all_names),
            out_names=tuple(out_names),
            lowering_input_output_aliases=(),
            sim_require_finite=True,
            sim_require_nnan=True,
            nc=nc,
        ))

    def _body(*args):
        ins, bufs = args[:n_params], args[n_params:]
        out = _call(ins, bufs)
        for _ in range(repeat - 1):
            # chain on previous outputs: serializes executions on-device so
            # one host dispatch amortizes over `repeat` NEFF runs
            out = _call(ins, out)
        return out

    devices = jax.devices()[:NCORES]
    mesh = Mesh(np.asarray(devices), ("core",))
    nin = n_params + len(out_names)
    f = jax.jit(
        shard_map(
            _body, mesh=mesh,
            in_specs=(PartitionSpec("core"),) * nin,
            out_specs=(PartitionSpec("core"),) * len(out_names),
            check_rep=False,
        ),
        keep_unused=True,
    )
    sharding = jax.sharding.NamedSharding(mesh, PartitionSpec("core"))
    concat_in = [
        jax.device_put(
            np.concatenate([np.asarray(in_maps[c][nm]) for c in range(NCORES)], axis=0),
            sharding)
        for nm in in_names
    ]
    concat_zeros = [
        jax.device_put(
            np.zeros((NCORES * av.shape[0], *av.shape[1:]), av.dtype), sharding)
        for av in out_avals
    ]
    args = concat_in + concat_zeros

    out = f(*args)
    jax.block_until_ready(out)
    t0 = _time.perf_counter()
    for _ in range(iters):
        out = f(*args)
    jax.block_until_ready(out)
    dt = (_time.perf_counter() - t0) / iters
    results = [
        {nm: np.asarray(out[i]).reshape(NCORES, *out_avals[i].shape)[c]
         for i, nm in enumerate(out_names)}
        for c in range(NCORES)
    ]
    return dt, results



# revision 19
# speedup vs baseline: 8.3579x; 8.3579x over previous
"""Trainium2 Bass kernel for nn_CPAMDec_Mix (dual cross-attention mix block).

Math (per batch b):
    q1 = wq1 @ x1      q2 = wq2 @ x2          (1x1 convs, [128, N] each)
    qT = concat(q1, q2) on channel -> [256, N]
    k_sT = w k_s @ y_s^T                      ([256, K])
    v_s  = y_s @ wv_s^T                       ([K, C])
    e_sT[k, n] = sum_d k_sT[d, k] qT[d, n]    ([K, N])
    attnT = softmax_k(|e1T - e2T|)            (softmax over k, no max-sub:
                                               |e| <= ~40 << 88 overflow)
    out_s = scale * (v_s^T @ attnT + bv_s) + x_s

Sharding: data-parallel over batch B=16 across 8 cores (2 batches/core),
weights replicated. Everything stays in [c, n] layout so DRAM I/O is
contiguous; softmax lives in [k, n] layout so no transposes are needed
(k-sum via ones-matmul, 1/sum broadcast comes out of the same matmul).
"""

import os
import numpy as np

import concourse.mybir as mybir
import concourse.tile as tile
from concourse import bacc
from concourse.bass import ts
from concourse.bass_utils import run_bass_kernel_spmd

F32 = mybir.dt.float32
F32R = mybir.dt.float32r
AF = mybir.ActivationFunctionType
ALU = mybir.AluOpType

B, C, WH, K = 16, 512, 4096, 128
NCORES = 8
BPC = B // NCORES          # batches per core
D = 128                    # per-stream q channels (C // 4)
NT = int(os.environ.get("KM_NT", 512))   # n-tile size
NTILES = WH // NT
CCH = C // 128             # 4 c-chunks

# Matmul input dtype knobs (float32r = single-pass PE fp32, 4x faster at
# moving-dim >= 256; float32 = 2-pass full precision).
DT_Q = F32R   # q projections (also x-tile dtype)
DT_E = F32    # attention logits (kT / q tile dtype)
DT_V = F32R   # v + k projections (y / wk / wv tile dtype)
DT_O = F32R   # output projection (vv / attnt tile dtype)
DT_S = F32R   # softmax denominator ones-matmul (ones / expt tile dtype)

_PROGRAM = None
LAST_RESULTS = None


def _body(tc, io):
    nc = tc.nc
    from contextlib import ExitStack

    with ExitStack() as ctx:
        def _bufs(name, default):
            return int(os.environ.get(f"KM_BUFS_{name}", default))

        consts = ctx.enter_context(tc.tile_pool(name="consts", bufs=1))
        bpool = ctx.enter_context(tc.tile_pool(name="batch", bufs=2))
        xpool = ctx.enter_context(tc.tile_pool(name="xs", bufs=_bufs("X", 4)))
        qpool = ctx.enter_context(tc.tile_pool(name="qs", bufs=_bufs("Q", 2)))
        spool = ctx.enter_context(tc.tile_pool(name="soft", bufs=_bufs("S", 2)))
        opool = ctx.enter_context(tc.tile_pool(name="outs", bufs=_bufs("O", 3)))
        pq = ctx.enter_context(tc.tile_pool(name="pq", bufs=_bufs("PQ", 1), space="PSUM"))
        pe = ctx.enter_context(tc.tile_pool(name="pe", bufs=_bufs("PE", 2), space="PSUM"))
        psb = ctx.enter_context(tc.tile_pool(name="psb", bufs=1, space="PSUM"))
        po = ctx.enter_context(tc.tile_pool(name="po", bufs=_bufs("PO", 4), space="PSUM"))

        # ---- constants (weights replicated per core) ----
        wq_sb, wk_sb, wv_sb, bq_sb, bk_sb, sbv_sb = {}, {}, {}, {}, {}, {}
        for s in (1, 2):
            wq_sb[s] = consts.tile([128, CCH, D], DT_Q, tag=f"wq{s}", name=f"wq{s}")
            nc.sync.dma_start(wq_sb[s][:], io[f"wq{s}t"][:].bitcast(DT_Q))
            wk_sb[s] = consts.tile([128, CCH, 2 * D], DT_V, tag=f"wk{s}", name=f"wk{s}")
            nc.sync.dma_start(wk_sb[s][:], io[f"wk{s}t"][:].bitcast(DT_V))
            wv_sb[s] = consts.tile([128, CCH, C], DT_V, tag=f"wv{s}", name=f"wv{s}")
            nc.sync.dma_start(wv_sb[s][:], io[f"wv{s}t"][:].bitcast(DT_V))
            bq_sb[s] = consts.tile([128, 1], F32, tag=f"bq{s}", name=f"bq{s}")
            nc.sync.dma_start(bq_sb[s][:], io[f"bq{s}"][:])
            bk_sb[s] = consts.tile([128, 2], F32, tag=f"bk{s}", name=f"bk{s}")
            nc.sync.dma_start(bk_sb[s][:], io[f"bk{s}"][:])
            sbv_sb[s] = consts.tile([128, CCH], F32, tag=f"sbv{s}", name=f"sbv{s}")
            nc.sync.dma_start(sbv_sb[s][:], io[f"sbv{s}"][:])
        scale_sb = consts.tile([128, 1], F32, tag="scale")
        nc.sync.dma_start(scale_sb[:], io["scale_rep"][:])
        ones_sb = consts.tile([128, 128], DT_S, tag="ones")
        nc.sync.dma_start(ones_sb[:], io["ones"][:].bitcast(DT_S))

        for _rep in range(int(os.environ.get("KM_REPEAT", 1))):
         for b in range(BPC):
            # ---- per-batch: k_sT [128, 2, 128] and v_s [128, C] ----
            kT, vv = {}, {}
            for s in (1, 2):
                yt = bpool.tile([128, CCH, K], DT_V, tag=f"y{s}", name=f"y{s}")
                nc.sync.dma_start(yt[:], io[f"y{s}t"][b].bitcast(DT_V))
                kT[s] = bpool.tile([128, 2, K], DT_E, tag=f"k{s}", name=f"k{s}")
                for dc in range(2):
                    pk = pe.tile([128, NT], F32, tag="pe1", name="pk")[:, :K]
                    for cc in range(CCH):
                        nc.tensor.matmul(
                            pk[:],
                            wk_sb[s][:, cc, ts(dc, D)],
                            yt[:, cc, :],
                            start=(cc == 0),
                            stop=(cc == CCH - 1),
                        )
                    # stream 2 is negated (and its bias host-negated) so the
                    # e-matmuls can accumulate e1 - e2 in one PSUM bank
                    nc.scalar.activation(
                        kT[s][:, dc, :], pk[:], AF.Identity,
                        bias=bk_sb[s][:, dc : dc + 1],
                        scale=(1.0 if s == 1 else -1.0),
                    )
                vv[s] = bpool.tile([128, C], DT_O, tag=f"v{s}", name=f"v{s}")
                pv = po.tile([128, C], F32, tag="po", name="pv")
                for cc in range(CCH):
                    nc.tensor.matmul(
                        pv[:],
                        yt[:, cc, :],
                        wv_sb[s][:, cc, :],
                        start=(cc == 0),
                        stop=(cc == CCH - 1),
                    )
                # v_s scaled by `scale` here; bias bv folded into the output
                # residual (attn rows sum to 1).
                nc.scalar.activation(
                    vv[s][:], pv[:], AF.Copy, bias=0.0, scale=scale_sb[:],
                )

            x_ap = {s: io[f"x{s}"][b].rearrange("(co p) n -> p co n", p=128)
                    for s in (1, 2)}
            o_ap = {s: io[f"out{s}"][b].rearrange("(co p) n -> p co n", p=128)
                    for s in (1, 2)}

            for nt in range(int(os.environ.get("KM_NTILES", NTILES))):
                nsl = ts(nt, NT)
                # ---- load x tiles ----
                xt = {}
                for s in (1, 2):
                    xt[s] = xpool.tile([128, CCH, NT], DT_Q, tag=f"x{s}", name=f"x{s}")
                    xeng = nc.scalar if (s == 2 and os.environ.get("KM_RING", "1") in ("2", "3")) else nc.sync
                    xeng.dma_start(xt[s][:], x_ap[s][:, :, nsl].bitcast(DT_Q))

                # ---- q projections: qT chunk s -> [128, NT] ----
                q = {}
                for s in (1, 2):
                    pqt = pq.tile([128, NT], F32, tag="pq", name="pqt")
                    for cc in range(CCH):
                        nc.tensor.matmul(
                            pqt[:],
                            wq_sb[s][:, cc, :],
                            xt[s][:, cc, :],
                            start=(cc == 0),
                            stop=(cc == CCH - 1),
                        )
                    q[s] = qpool.tile([128, NT], DT_E, tag=f"q{s}", name=f"q{s}")
                    nc.scalar.activation(
                        q[s][:], pqt[:], AF.Identity, bias=bq_sb[s][:],
                    )

                # ---- attention logit diff e1T - e2T, accumulated in PSUM
                # (kT[2] is pre-negated, so all 4 matmuls add into one bank) ----
                pdiff = pe.tile([128, NT], F32, tag="pe1", name="pdiff")
                for s in (1, 2):
                    for dc in range(2):
                        nc.tensor.matmul(
                            pdiff[:],
                            kT[s][:, dc, :],
                            q[dc + 1][:],
                            start=(s == 1 and dc == 0),
                            stop=(s == 2 and dc == 1),
                        )

                # ---- softmax over k (partition dim), no max subtraction ----
                adiff = spool.tile([128, NT], F32, tag="adiff")
                nc.scalar.activation(adiff[:], pdiff[:], AF.Abs)
                expt = spool.tile([128, NT], DT_S, tag="expt")
                nc.scalar.activation(expt[:], adiff[:], AF.Exp)
                # all-partition sum broadcast: ones[128,128]^T @ expt
                psum_s = psb.tile([128, NT], F32, tag="psb", name="psum_s")
                nc.tensor.matmul(
                    psum_s[:], ones_sb[:], expt[:],
                )
                rb = spool.tile([128, NT], F32, tag="rb")
                nc.vector.reciprocal(rb[:], psum_s[:])
                attnt = spool.tile([128, NT], DT_O, tag="attnt")
                nc.vector.tensor_mul(attnt[:], expt[:], rb[:])

                # ---- outputs: out_s[c, n] = v_s^T @ attnT + s*bv_s + x_s ----
                for s in (1, 2):
                    ot = opool.tile([128, CCH, NT], F32, tag=f"o{s}", name=f"o{s}")
                    for cc in range(CCH):
                        pot = po.tile([128, NT], F32, tag="po", name="pot")
                        nc.tensor.matmul(
                            pot[:],
                            vv[s][:, ts(cc, 128)],
                            attnt[:],
                        )
                        nc.vector.scalar_tensor_tensor(
                            ot[:, cc, :],
                            pot[:],
                            sbv_sb[s][:, cc : cc + 1],
                            xt[s][:, cc, :],
                            op0=ALU.add,
                            op1=ALU.add,
                        )
                    oeng = nc.scalar if os.environ.get("KM_RING", "1") in ("1", "3") else nc.sync
                    oeng.dma_start(o_ap[s][:, :, nsl], ot[:])


def build_program():
    nc = bacc.Bacc(
        "TRN2", target_bir_lowering=False, debug=False, enable_asserts=False,
    )
    io = {}

    def din(name, shape):
        io[name] = nc.dram_tensor(name, shape, F32, kind="ExternalInput").ap()

    def dout(name, shape):
        io[name] = nc.dram_tensor(name, shape, F32, kind="ExternalOutput").ap()

    din("x1", [BPC, C, WH])
    din("x2", [BPC, C, WH])
    din("y1t", [BPC, 128, CCH, K])
    din("y2t", [BPC, 128, CCH, K])
    for s in (1, 2):
        din(f"wq{s}t", [128, CCH, D])
        din(f"wk{s}t", [128, CCH, 2 * D])
        din(f"wv{s}t", [128, CCH, C])
        din(f"bq{s}", [128, 1])
        din(f"bk{s}", [128, 2])
        din(f"sbv{s}", [128, CCH])
    din("scale_rep", [128, 1])
    din("ones", [128, 128])
    dout("out1", [BPC, C, WH])
    dout("out2", [BPC, C, WH])

    with tile.TileContext(nc) as tc:
        _body(tc, io)
    nc.compile()
    return nc


def _get_program():
    global _PROGRAM
    if _PROGRAM is None:
        _PROGRAM = build_program()
    return _PROGRAM


def _to_chunked(w):
    # host weight [out, in] -> transposed chunked SBUF layout [p, co, out]
    # (wT[c, out] with input-channel c = co*128 + p), contiguous for DMA
    out_dim, in_dim = w.shape
    return np.ascontiguousarray(
        w.T.reshape(in_dim // 128, 128, out_dim).transpose(1, 0, 2)
    )


def _bias_chunks(bv):
    # [d] -> [128, d//128] with d = dc*128 + p
    return np.ascontiguousarray(bv.reshape(-1, 128).T)


def prepare_in_maps(inputs):
    f = lambda a: np.ascontiguousarray(np.asarray(a, dtype=np.float32))
    x1 = f(inputs["x1"]).reshape(B, C, WH)
    x2 = f(inputs["x2"]).reshape(B, C, WH)
    # y^T per batch in chunked layout [b, p, co, k]
    def yt_chunk(y):
        ytr = f(y).transpose(0, 2, 1)  # [B, C, K]
        return np.ascontiguousarray(
            ytr.reshape(B, CCH, 128, K).transpose(0, 2, 1, 3)
        )
    y1t = yt_chunk(inputs["y1"])
    y2t = yt_chunk(inputs["y2"])
    scale = float(np.asarray(inputs["scale"]).reshape(-1)[0])

    shared = {"scale_rep": np.full((128, 1), scale, np.float32),
              "ones": np.ones((128, 128), np.float32)}
    for s in (1, 2):
        shared[f"wq{s}t"] = _to_chunked(f(inputs[f"wq{s}"]))
        shared[f"wk{s}t"] = _to_chunked(f(inputs[f"wk{s}"]))
        shared[f"wv{s}t"] = _to_chunked(f(inputs[f"wv{s}"]))
        shared[f"bq{s}"] = f(inputs[f"bq{s}"]).reshape(128, 1)
        bk_sign = 1.0 if s == 1 else -1.0
        shared[f"bk{s}"] = _bias_chunks(bk_sign * f(inputs[f"bk{s}"]))
        shared[f"sbv{s}"] = _bias_chunks(scale * f(inputs[f"bv{s}"]))

    in_maps = []
    for c in range(NCORES):
        sl = slice(BPC * c, BPC * (c + 1))
        in_maps.append({
            "x1": np.ascontiguousarray(x1[sl]),
            "x2": np.ascontiguousarray(x2[sl]),
            "y1t": np.ascontiguousarray(y1t[sl]),
            "y2t": np.ascontiguousarray(y2t[sl]),
            **shared,
        })
    return in_maps


def kernel(**inputs):
    global LAST_RESULTS
    nc = _get_program()
    in_maps = prepare_in_maps(inputs)
    try:
        res = run_bass_kernel_spmd(nc, in_maps, list(range(NCORES)))
    except Exception:
        # transient NRT device hiccups have been observed; retry once
        res = run_bass_kernel_spmd(nc, in_maps, list(range(NCORES)))
    LAST_RESULTS = res
    out1 = np.concatenate(
        [res.results[c]["out1"] for c in range(NCORES)], axis=0
    ).reshape(B, C, 64, 64)
    out2 = np.concatenate(
        [res.results[c]["out2"] for c in range(NCORES)], axis=0
    ).reshape(B, C, 64, 64)
    return out1, out2


def bench(inputs, iters=30, repeat=1, nc=None):
    """Time warm back-to-back executions of the compiled NEFF on 8 cores.

    Replicates run_bass_via_pjrt's shard_map jit, but without output-buffer
    donation so device-resident inputs can be reused across calls (this
    kernel writes every output element, so uninitialized result buffers are
    fine). Returns (per_call_seconds, results_list).
    """
    import time as _time
    import jax
    import concourse.mybir as _mybir
    from jax.experimental.shard_map import shard_map
    from jax.sharding import Mesh, PartitionSpec
    from concourse.bass2jax import _bass_exec_p, install_neuronx_cc_hook

    from concourse.bass2jax import partition_id_tensor
    install_neuronx_cc_hook()
    if nc is None:
        nc = _get_program()
    in_maps = prepare_in_maps(inputs)

    partition_name = nc.partition_id_tensor.name if nc.partition_id_tensor else None
    in_names, out_names, out_avals = [], [], []
    for alloc in nc.m.functions[0].allocations:
        if not isinstance(alloc, _mybir.MemoryLocationSet):
            continue
        name = alloc.memorylocations[0].name
        if alloc.kind == "ExternalInput":
            if name != partition_name:
                in_names.append(name)
        elif alloc.kind == "ExternalOutput":
            out_names.append(name)
            out_avals.append(jax.core.ShapedArray(
                tuple(alloc.tensor_shape), _mybir.dt.np(alloc.dtype)))
    n_params = len(in_names)
    all_names = in_names + out_names
    if partition_name is not None:
        all_names = all_names + [partition_name]

    def _call(ins, bufs):
        operands = list(ins) + list(bufs)
        if partition_name is not None:
            operands.append(partition_id_tensor())
        return tuple(_bass_exec_p.bind(
            *operands,
            out_avals=tuple(out_avals),
            in_names=tuple(all_names),
            out_names=tuple(out_names),
            lowering_input_output_aliases=(),
            sim_require_finite=True,
            sim_require_nnan=True,
            nc=nc,
        ))

    def _body(*args):
        ins, bufs = args[:n_params], args[n_params:]
        out = _call(ins, bufs)
        for _ in range(repeat - 1):
            # chain on previous outputs: serializes executions on-device so
            # one host dispatch amortizes over `repeat` NEFF runs
            out = _call(ins, out)
        return out

    devices = jax.devices()[:NCORES]
    mesh = Mesh(np.asarray(devices), ("core",))
    nin = n_params + len(out_names)
    f = jax.jit(
        shard_map(
            _body, mesh=mesh,
            in_specs=(PartitionSpec("core"),) * nin,
            out_specs=(PartitionSpec("core"),) * len(out_names),
            check_rep=False,
        ),
        keep_unused=True,
    )
    sharding = jax.sharding.NamedSharding(mesh, PartitionSpec("core"))
    concat_in = [
        jax.device_put(
            np.concatenate([np.asarray(in_maps[c][nm]) for c in range(NCORES)], axis=0),
            sharding)
        for nm in in_names
    ]
    concat_zeros = [
        jax.device_put(
            np.zeros((NCORES * av.shape[0], *av.shape[1:]), av.dtype), sharding)
        for av in out_avals
    ]
    args = concat_in + concat_zeros

    out = f(*args)
    jax.block_until_ready(out)
    t0 = _time.perf_counter()
    for _ in range(iters):
        out = f(*args)
    jax.block_until_ready(out)
    dt = (_time.perf_counter() - t0) / iters
    results = [
        {nm: np.asarray(out[i]).reshape(NCORES, *out_avals[i].shape)[c]
         for i, nm in enumerate(out_names)}
        for c in range(NCORES)
    ]
    return dt, results

